# revision 10
# baseline (speedup 1.0000x reference)
"""Multi-head attention (B=2, S=2048, EMB=1024, H=16) on 8 Trainium2 cores.

v4 -- hybrid batch x head sharding: core c owns batch c//4 and heads
{4g..4g+3} where g = c%4 (a 256-wide feature slice, two 128-wide head
PAIRS).  Each core projects ONLY its batch's 2048 tokens (halving HBM
reads vs the head-only split: 14 MB vs 26 MB) and writes a [2048, 1024]
bf16 partial output (4 MB); the host sums 4 partials per batch.

Per-core steady state is ACT(exp)-bound: 128 chunks x [128 keys,
2 heads x 512 q] exp at ~1.15us each = ~147us of scalar-engine time.
The PE work per chunk (score quad ~215ns + att pair ~430ns + ~2 windows
of projection fillers) fits inside the exp period, so the design goal is
a gapless ACT pipeline:

  - scores_T [k, q] per key-chunk via the 2x2 tile_position quad
    (2 heads x 2 key-halves, K=64 M=64 N=512, concurrent quadrants)
  - exp: one ACT call per chunk ([128, 1024] over a 2-bank PSUM tile,
    scale=1/8, no max subtraction; |scores|/8 < ~7 stays in range)
  - att: M=65 (64 V dims + ones column accumulating the softmax
    denominator), the two head MMs emitted ADJACENT so they pipeline at
    stream rate
  - V projected token-major with N=256 matmuls (both head pairs share
    one psum bank), 2x the efficiency of per-pair N=128 MMs
  - 10 garbage warm-up matmuls at t=0 lift the PE HAM clock gate
    (4/8 -> 8/8) before the first real projection; a dummy exp at t=0
    triggers the ~2.7us ACT table load under the DMA staging shadow
  - x-block DMAs issued in need order; filler units carry min_step
    gates so the in-order PE queue never blocks on an unlanded DMA
"""

from collections import deque
from contextlib import ExitStack

import numpy as np
import ml_dtypes

import concourse.bass as bass  # noqa: F401
import concourse.mybir as mybir
import concourse.tile as tile
from concourse import bacc
from concourse.bass_utils import run_bass_kernel_spmd

BF = mybir.dt.bfloat16
F32 = mybir.dt.float32

EMB = 1024
HEADS = 16
HD = EMB // HEADS          # 64
B, S = 2, 2048
N_CORES = 8
P = 128
NE = EMB // P              # 8 contraction chunks
FEATS = 256                # features per core (4 heads)
PAIRS = 2                  # head pairs per core
TOKS = S                   # tokens per core (its batch)
QB = 512                   # query block (psum bank width)
NQ = TOKS // QB            # 4 qblocks
NB = TOKS // QB            # 4 x-blocks per tensor
SB = TOKS // P             # 16 key chunks
NCH = PAIRS * NQ * SB      # 128 chunks
EXPF = mybir.ActivationFunctionType.Exp
SCALE = 1.0 / np.sqrt(HD)  # 0.125


def _build_nc():
    nc = bacc.Bacc(num_devices=N_CORES)
    dp = nc.declare_dram_parameter
    xq = dp("xq", [NB, P, NE, QB], BF, isOutput=False)
    xk = dp("xk", [NB, P, NE, QB], BF, isOutput=False)
    xv = dp("xv", [NB, P, NE, QB], BF, isOutput=False)
    WqT = dp("WqT", [P, NE, FEATS], BF, isOutput=False)
    WkT = dp("WkT", [P, NE, FEATS], BF, isOutput=False)
    WvT = dp("WvT", [P, NE, FEATS], BF, isOutput=False)
    WoT = dp("WoT", [P, PAIRS, EMB], BF, isOutput=False)
    bqp = dp("bqp", [P, PAIRS], F32, isOutput=False)
    bkp = dp("bkp", [P, PAIRS], F32, isOutput=False)
    bvp = dp("bvp", [P, PAIRS], F32, isOutput=False)
    out = dp("out", [TOKS, EMB], BF, isOutput=True)

    with tile.TileContext(nc) as tc, ExitStack() as ctx:
        wpool = ctx.enter_context(tc.tile_pool(name="wts", bufs=1))
        apool = ctx.enter_context(tc.tile_pool(name="acts", bufs=1))
        xpool = ctx.enter_context(tc.tile_pool(name="xin", bufs=12))
        ppool = ctx.enter_context(tc.tile_pool(name="probs", bufs=7))
        dpool = ctx.enter_context(tc.tile_pool(name="dn", bufs=2))
        opool = ctx.enter_context(tc.tile_pool(name="ob", bufs=2))
        genps = ctx.enter_context(tc.tile_pool(name="gen", bufs=2, space="PSUM"))
        scps = ctx.enter_context(tc.tile_pool(name="sc", bufs=2, space="PSUM"))
        attps = ctx.enter_context(tc.tile_pool(name="att", bufs=1, space="PSUM"))

        qhT = apool.tile([P, PAIRS, TOKS], BF, tag="qhT")
        khT = apool.tile([P, PAIRS, TOKS], BF, tag="khT")
        vhe = apool.tile([P, SB, PAIRS, 2, HD + 1], BF, tag="vhe")
        attT = apool.tile([P, PAIRS, TOKS], BF, tag="attT")

        Wq_sb = wpool.tile([P, NE, FEATS], BF, tag="WqT")
        Wk_sb = wpool.tile([P, NE, FEATS], BF, tag="WkT")
        Wv_sb = wpool.tile([P, NE, FEATS], BF, tag="WvT")
        Wo_sb = wpool.tile([P, PAIRS, EMB], BF, tag="WoT")
        bq_sb = wpool.tile([P, PAIRS], F32, tag="bqp")
        bk_sb = wpool.tile([P, PAIRS], F32, tag="bkp")
        bv_sb = wpool.tile([P, PAIRS], F32, tag="bvp")
        warm = wpool.tile([P, QB], BF, tag="warm")
        ones64 = wpool.tile([1, HD], F32, tag="ones64")

        # t=0 work with no DMA deps: ACT table load fires under the DMA
        # shadow; garbage matmuls lift the HAM clock gate (~3.4us of PE
        # activity) so the first real projections run at 2.4 GHz
        nc.vector.memset(warm[:], 0.0)
        nc.vector.memset(ones64[:], 1.0)
        dmy = dpool.tile([P, 8], BF, tag="dmy")
        nc.scalar.activation(dmy[:], warm[:, 0:8], EXPF, scale=SCALE)
        nc.vector.memset(vhe[:, :, :, :, HD], 1.0)

        # ---- input staging, one queue, in need order; the first k/q
        # blocks are split so the first projections pipeline behind the
        # DMA at kk-half granularity (Tile tracks subtile deps) ----
        xblocks = {}

        def stage_block(name, src, nb, split=False):
            xb = xpool.tile([P, NE, QB], BF, tag="xin", name=f"x_{name}{nb}")
            if split:
                nc.sync.dma_start(xb[:, 0:4, :], src[nb, :, 0:4, :])
                nc.sync.dma_start(xb[:, 4:8, :], src[nb, :, 4:8, :])
            else:
                nc.sync.dma_start(xb[:], src[nb, :, :, :])
            xblocks[(name, nb)] = xb

        nc.sync.dma_start(Wk_sb[:], WkT[:])
        nc.sync.dma_start(Wq_sb[:], WqT[:])
        stage_block("k", xk, 0, split=True)
        stage_block("q", xq, 0, split=True)
        nc.sync.dma_start(bq_sb[:], bqp[:])
        nc.sync.dma_start(bk_sb[:], bkp[:])
        stage_block("k", xk, 1)
        nc.sync.dma_start(Wv_sb[:], WvT[:])
        nc.sync.dma_start(bv_sb[:], bvp[:])
        stage_block("v", xv, 0)
        stage_block("k", xk, 2)
        stage_block("v", xv, 1)
        stage_block("k", xk, 3)
        stage_block("v", xv, 2)
        stage_block("q", xq, 1)
        stage_block("v", xv, 3)
        stage_block("q", xq, 2)
        stage_block("q", xq, 3)
        nc.sync.dma_start(Wo_sb[:], WoT[:])

        # garbage matmuls with no DMA deps: keep the PE busy from t=0 so
        # the HAM clock gate opens (~3.4us) and STAYS open until the first
        # real projection's inputs land (~10us)
        for _ in range(16):
            nc.tensor.matmul(
                genps.tile([P, QB], F32, tag="gen", name="warmps")[:],
                warm[:, 0:P], warm[:], start=True, stop=True,
            )

        # ---- emission helpers ----
        def qk_half(dst, W_sb, xname, b_sb, r, nb, half, box):
            """Half of one pair's 512-token q/k projection block."""
            xb = xblocks[(xname, nb)]
            if half == 0:
                box[0] = genps.tile(
                    [P, QB], F32, tag="gen", name=f"pj_{xname}_{r}_{nb}"
                )
            ps = box[0]
            for kk in range(half * 4, half * 4 + 4):
                nc.tensor.matmul(
                    ps[:],
                    W_sb[:, kk, r * P : (r + 1) * P],
                    xb[:, kk, :],
                    start=(kk == 0),
                    stop=(kk == NE - 1),
                )
            if half == 1:
                t0 = nb * QB
                nc.vector.tensor_scalar_add(
                    dst[:, r, t0 : t0 + QB], ps[:], b_sb[:, r : r + 1]
                )

        def v_half(c, half, box):
            """Token-major v projection of one 128-token chunk (N=256 MMs
            cover BOTH head pairs in a single psum bank)."""
            nb, ci = divmod(c, NB)
            xb = xblocks[("v", nb)]
            if half == 0:
                box[0] = genps.tile([P, QB], F32, tag="gen", name=f"pv_{c}")
            ps = box[0]
            for kk in range(half * 4, half * 4 + 4):
                nc.tensor.matmul(
                    ps[:, 0:FEATS],
                    xb[:, kk, ci * P : (ci + 1) * P],
                    Wv_sb[:, kk, :],
                    start=(kk == 0),
                    stop=(kk == NE - 1),
                )
            if half == 1:
                nc.vector.tensor_copy(
                    vhe[:, c, :, :, 0:HD],
                    ps[:, 0:FEATS].rearrange(
                        "p (r h d) -> p r h d", r=PAIRS, d=HD
                    ),
                )

        def v_chunk(c):
            for f in unit_halves(v_half, c):
                f()

        def unit_halves(fn, *args, wait=None):
            # wait: absolute sim-time hint (us) fed to the Tile scheduler so
            # it does not slot this unit's PE work ahead of critical-path
            # matmuls before the unit's input DMA has actually landed
            # (head-of-line blocking in the in-order PE queue)
            box = [None]

            def mk(h):
                def g():
                    if wait is None:
                        fn(*args, h, box)
                    else:
                        with tc.tile_wait_until(wait / 1000.0):
                            fn(*args, h, box)
                return g

            return [mk(h) for h in range(2)]

        prio: deque = deque()     # normalize tails + out-proj: no DMA deps
        fillers: deque = deque()  # (min_step, fn): gated on x-block arrival
        T0 = 11.0                 # sim-time of chunk 0 (us, body-relative)
        PERIOD = 1.31             # per-chunk exp period (us)

        def slot(step, off=0.5):
            return T0 + PERIOD * step + off

        def fill(step, budget=1):
            done = 0
            with tc.tile_wait_until(slot(step) / 1000.0):
                while done < budget and prio:
                    prio.popleft()()
                    done += 1
                while done < budget and fillers and fillers[0][0] <= step:
                    fillers.popleft()[1]()
                    done += 1

        # chunk list: pair-major, then qblock, then key chunk
        chunks = [
            (r, qi, j) for r in range(PAIRS) for qi in range(NQ)
            for j in range(SB)
        ]

        def sc_chunk(r, qi, j):
            """Scores+exp for one key chunk: 2x2 matmul quad, one ACT."""
            q0 = qi * QB
            j0 = j * P
            sc = scps.tile([P, 2 * QB], F32, tag="sc", name=f"sc_{r}_{qi}_{j}")
            nc.tensor.matmul(
                sc[0:HD, 0:QB],
                khT[0:HD, r, j0 : j0 + HD],
                qhT[0:HD, r, q0 : q0 + QB],
                start=True, stop=True,
            )
            nc.tensor.matmul(
                sc[HD:P, 0:QB],
                khT[0:HD, r, j0 + HD : j0 + P],
                qhT[0:HD, r, q0 : q0 + QB],
                start=True, stop=True,
            )
            nc.tensor.matmul(
                sc[0:HD, QB:],
                khT[HD:P, r, j0 : j0 + HD],
                qhT[HD:P, r, q0 : q0 + QB],
                start=True, stop=True,
            )
            nc.tensor.matmul(
                sc[HD:P, QB:],
                khT[HD:P, r, j0 + HD : j0 + P],
                qhT[HD:P, r, q0 : q0 + QB],
                start=True, stop=True,
            )
            pr = ppool.tile([P, 2 * QB], BF, tag="pr", name=f"pr_{r}_{qi}_{j}")
            nc.scalar.activation(pr[:], sc[:], EXPF, scale=SCALE)
            return pr

        att_tiles = {}

        def get_att(r, qi):
            if (r, qi) not in att_tiles:
                att_tiles[(r, qi)] = attps.tile(
                    [HD + 1, 2, QB], F32, tag="att", name=f"att_{r}_{qi}"
                )
            return att_tiles[(r, qi)]

        def normalize_block(r, qi, attAB, last=False):
            # one copy drains the att psum fast (its slot is WAR-waited by
            # the next qblock's first att matmul in the in-order PE queue)
            q0 = qi * QB
            if last:
                au2 = attAB
            else:
                au2 = dpool.tile([HD + 1, 2, QB], F32, tag="au",
                                 name=f"au_{r}_{qi}")
                with tc.high_priority():
                    nc.vector.tensor_copy(au2[:, 0, :], attAB[:, 0, :])
                    nc.vector.tensor_copy(au2[:, 1, :], attAB[:, 1, :])

            def tail(h):
                d0 = dpool.tile([1, QB], F32, tag=f"d0{h}",
                                name=f"d0_{r}_{qi}_{h}")
                nc.vector.tensor_copy(d0[:], au2[HD : HD + 1, h, :])
                r0 = dpool.tile([1, QB], F32, tag=f"r0{h}",
                                name=f"r0_{r}_{qi}_{h}")
                nc.vector.reciprocal_approx_fast(r0[:], d0[:])
                rb = dpool.tile([HD, QB], F32, tag=f"rb{h}",
                                name=f"rb_{r}_{qi}_{h}")
                nc.gpsimd.partition_broadcast(rb[:], r0[:])
                nc.vector.tensor_mul(
                    attT[h * HD : (h + 1) * HD, r, q0 : q0 + QB],
                    au2[0:HD, h, :],
                    rb[:],
                )

            if last:
                tail(0)
                tail(1)
            else:
                prio.append(lambda: tail(0))
                prio.append(lambda: tail(1))

        tail_mode = [False]

        def outproj_half(qi, t, half, box):
            t0 = qi * QB + t * P
            if half == 0:
                box[0] = opool.tile([P, EMB], BF, tag="ob",
                                    name=f"ob_{qi}_{t}")
            ob = box[0]
            ps = genps.tile([P, QB], F32, tag="gen", name=f"o_{qi}_{t}_{half}")
            for r in range(PAIRS):
                nc.tensor.matmul(
                    ps[:],
                    attT[:, r, t0 : t0 + P],
                    Wo_sb[:, r, half * QB : (half + 1) * QB],
                    start=(r == 0),
                    stop=(r == PAIRS - 1),
                )
            if tail_mode[0]:
                nc.scalar.copy(ob[:, half * QB : (half + 1) * QB], ps[:])
            else:
                nc.vector.tensor_copy(
                    ob[:, half * QB : (half + 1) * QB], ps[:]
                )
            if half == 1:
                nc.gpsimd.dma_start(out[t0 : t0 + P, :], ob[:])

        def push_outproj(qi):
            for t in range(QB // P):
                for f in unit_halves(outproj_half, qi, t):
                    prio.append(f)

        def att_half(i, h, pr):
            r, qi, j = chunks[i]
            attAB = get_att(r, qi)
            nc.tensor.matmul(
                attAB[:, h, :],
                vhe[:, j, r, h, :],
                pr[:, h * QB : (h + 1) * QB],
                start=(j == 0),
                stop=(j == SB - 1),
            )
            if h == 1 and j == SB - 1:
                last = i == NCH - 1
                normalize_block(r, qi, attAB, last=last)
                del att_tiles[(r, qi)]
                if r == PAIRS - 1:
                    if last:
                        for t in range(QB // P):
                            for f in unit_halves(outproj_half, qi, t):
                                prio.append(f)
                    else:
                        push_outproj(qi)

        # ---- head: minimal serial chain to the first exp ----
        for f in unit_halves(qk_half, khT, Wk_sb, "k", bk_sb, 0, 0,
                             wait=5.2):
            f()
        for f in unit_halves(qk_half, qhT, Wq_sb, "q", bq_sb, 0, 0,
                             wait=8.2):
            f()

        # filler queue: min_step approximates when the unit's x-block DMA
        # has landed (1 step ~ 1.15us of attention; step 0 ~ 10.5us abs)
        # (min_step, unit): min_step paces EMISSION; the wait= hint tells
        # the Tile scheduler the unit's real DMA arrival time (us) so its
        # static reordering never puts a blocked unit ahead of the
        # exp-critical quad stream
        pend = []
        arr_k = {1: 12.7, 2: 20.1, 3: 26.1}
        arr_v = {0: 17.2, 1: 23.1, 2: 29.0, 3: 35.0}
        arr_q = {1: 32.0, 2: 37.9, 3: 40.9}
        stp_k = {1: 3, 2: 8, 3: 12}
        stp_v = {0: 1, 1: 6, 2: 11, 3: 16}
        stp_q = {1: 14, 2: 20, 3: 22}
        for nb in range(1, NB):
            for hi, f in enumerate(unit_halves(
                    qk_half, khT, Wk_sb, "k", bk_sb, 0, nb,
                    wait=arr_k[nb])):
                pend.append((stp_k[nb], f))
        for c in range(2, SB):
            w = arr_v[c // NB] + 0.7 * (c % NB)
            for f in unit_halves(v_half, c, wait=w):
                pend.append((min(stp_v[c // NB], c), f))
        for nb in range(1, NB):
            for f in unit_halves(qk_half, qhT, Wq_sb, "q", bq_sb, 0, nb,
                                 wait=arr_q[nb]):
                pend.append((stp_q[nb], f))
        for nb in range(NB):
            for f in unit_halves(qk_half, khT, Wk_sb, "k", bk_sb, 1, nb,
                                 wait=44.0 + 2.6 * nb):
                pend.append((28 + 2 * nb, f))
        for nb in range(NB):
            for f in unit_halves(qk_half, qhT, Wq_sb, "q", bq_sb, 1, nb,
                                 wait=55.0 + 2.6 * nb):
                pend.append((40 + 2 * nb, f))
        for ms, f in sorted(pend, key=lambda e: e[0]):
            fillers.append((ms, f))

        # ---- main pipeline: att emission lags the quad stream by 2
        # chunks so the att-psum drain copy has two exp periods to clear
        # its WAR before the next qblock's first att matmul reaches the
        # in-order PE queue head ----
        sc_pr = {}
        with tc.tile_wait_until(slot(0, 0.0) / 1000.0):
            sc_pr[0] = sc_chunk(*chunks[0])
        for c in (0, 1):
            for f in unit_halves(v_half, c, wait=17.0 + 0.7 * c):
                f()
        for i in range(1, NCH):
            with tc.tile_wait_until(slot(i, 0.0) / 1000.0):
                sc_pr[i] = sc_chunk(*chunks[i])
                if i >= 2:
                    pr = sc_pr.pop(i - 2)
                    att_half(i - 2, 0, pr)
                    att_half(i - 2, 1, pr)
            if i % SB not in (SB - 1, 0):
                fill(i)
        for i in (NCH - 2, NCH - 1):
            pr = sc_pr.pop(i)
            if i == NCH - 1:
                tail_mode[0] = True
            att_half(i, 0, pr)
            att_half(i, 1, pr)
        for _ in range(6):
            nc.tensor.matmul(
                genps.tile([P, QB], F32, tag="gen", name="warmtail")[:],
                warm[:, 0:P], warm[:], start=True, stop=True,
            )
        while prio or fillers:
            fill(NCH, budget=4)

    nc.finalize()
    return nc


_NC_CACHE: list = [None]
_BO_CACHE: list = [None]


def _get_nc(*_args):
    if _NC_CACHE[0] is None:
        _NC_CACHE[0] = _build_nc()
    return _NC_CACHE[0]


def _feat_tiled(xT):
    """[EMB, n] -> [128, NE, n] contiguous (feature chunks on partitions)."""
    n = xT.shape[1]
    return np.ascontiguousarray(xT.reshape(NE, P, n).transpose(1, 0, 2))


def _stage(inputs):
    bf = ml_dtypes.bfloat16
    f32 = np.float32

    def arr(name):
        return np.asarray(inputs[name], f32)

    q, k, v = arr("q"), arr("k"), arr("v")
    Wq, Wk, Wv, Wo = arr("Wq"), arr("Wk"), arr("Wv"), arr("Wo")
    bq, bk, bv, bo = arr("bq"), arr("bk"), arr("bv"), arr("bo")
    _BO_CACHE[0] = bo if bool(np.any(bo)) else None

    def xt(x2d):  # [TOKS, EMB] -> [NB, 128, NE, QB] bf16 blocked
        xT = np.ascontiguousarray(x2d.T)  # [EMB, TOKS]
        blocks = [
            _feat_tiled(xT[:, i * QB : (i + 1) * QB]) for i in range(NB)
        ]
        return np.ascontiguousarray(np.stack(blocks)).astype(bf)

    xq_b = [xt(q[b]) for b in range(B)]
    xk_b = [xt(k[b]) for b in range(B)]
    xv_b = [xt(v[b]) for b in range(B)]

    in_maps = []
    for c in range(N_CORES):
        b = c // 4
        g = c % 4
        F = slice(g * FEATS, (g + 1) * FEATS)

        def bias_tiled(bvec):
            return np.ascontiguousarray(
                bvec[F].reshape(PAIRS, P).T
            ).astype(f32)

        m = {
            "xq": xq_b[b],
            "xk": xk_b[b],
            "xv": xv_b[b],
            "WqT": _feat_tiled(np.ascontiguousarray(Wq.T[:, F])).astype(bf),
            "WkT": _feat_tiled(np.ascontiguousarray(Wk.T[:, F])).astype(bf),
            "WvT": _feat_tiled(np.ascontiguousarray(Wv.T[:, F])).astype(bf),
            "WoT": np.ascontiguousarray(
                Wo.T[F, :].reshape(PAIRS, P, EMB).transpose(1, 0, 2)
            ).astype(bf),
            "bqp": bias_tiled(bq),
            "bkp": bias_tiled(bk),
            "bvp": bias_tiled(bv),
        }
        in_maps.append(m)
    return in_maps, True, True


def _assemble(results):
    full = np.empty((B, S, EMB), np.float32)
    for b in range(B):
        acc = results[4 * b]["out"].astype(np.float32)
        for g in range(1, 4):
            acc += results[4 * b + g]["out"].astype(np.float32)
        full[b] = acc
    if _BO_CACHE[0] is not None:
        full += _BO_CACHE[0]
    return full


def kernel(**inputs) -> np.ndarray:
    in_maps, _, _ = _stage(inputs)
    nc = _get_nc()
    res = run_bass_kernel_spmd(nc, in_maps, list(range(N_CORES)))
    return _assemble(res.results)


# revision 11
# speedup vs baseline: 1.0273x; 1.0273x over previous
"""Multi-head attention (B=2, S=2048, EMB=1024, H=16) on 8 Trainium2 cores.

v3 — tensor-parallel over heads: core c owns heads {2c, 2c+1} (a 128-wide
feature slice F_c of the QKV projections and of Wo's rows).  Each core:
  - projects q/k/v for ALL 4096 tokens through its 1024x128 weight slices
  - runs attention for its 2 heads over both batches
  - computes the PARTIAL output projection out_c = att_c @ Wo[:, F_c].T
The 8 partial outputs (bf16) are summed on the host (linear in Wo rows), so
no device collectives are needed and no compute is redundant: per-core FLOPs
drop from ~15 G (query-sharded baseline with 4x-redundant K/V projections)
to ~8.6 G (perfect 1/8 split of total work).

Layouts (all device matmuls contract over the partition dim):
  - inputs staged HOST-side as [8 blocks, 128, 8, 512] so every x-block DMA
    is one 8 KiB-contiguous run per partition (descriptor count, not
    bandwidth, limited an earlier cut)
  - qhT/khT [128 = (headA 64 | headB 64), 4096 tok] bf16, feature-major
  - vhe [128 tok, chunk, head, 65] bf16 token-major with an all-ones 65th
    column so the att matmul accumulates softmax denominators for free
  - scores_T [k, q] per key-chunk via a 2x2 tile_position quad: 4 matmuls
    (2 heads x 2 key-halves, K=64 M=64 N=512) occupy disjoint PE-array
    quadrants and run concurrently (measured 54 ns/MM vs 217 serial)
  - exp: one ACT call per chunk ([128, 1024] over a 2-bank PSUM tile,
    scale=1/8, no max subtraction; |scores|/8 < ~7 so exp stays in range)
  - normalize: copy denom row, gpsimd partition_broadcast, DVE
    reciprocal_approx_fast on [64, 512] (full-width lanes), multiply
  - out partial [tok, 1024] bf16: one DMA per 128-token chunk, issued from
    the gpsimd queue to keep descriptor generation off the sync engine

Pipelining: the score PSUM is double-buffered (scps bufs=2), so the quad of
chunk j+1 fills one 2-bank buffer while ACT exp-drains the other — ACT runs
gapless and the PE never idles long enough for the HAM clock gate to drop
the array to 1.2 GHz (the dominant loss in earlier cuts).  All other PE work
(v-projection of batch 0, every projection of batch 1, the out-projection of
the previous qblock) is queued as <=1.2us filler units, popped one per chunk
into the PE's ~0.5us-per-chunk slack.
"""

from collections import deque
from contextlib import ExitStack

import numpy as np
import ml_dtypes

import concourse.bass as bass  # noqa: F401
import concourse.mybir as mybir
import concourse.tile as tile
from concourse import bacc
from concourse.bass_utils import run_bass_kernel_spmd

BF = mybir.dt.bfloat16
F32 = mybir.dt.float32

EMB = 1024
HEADS = 16
HD = EMB // HEADS          # 64
B, S = 2, 2048
N_CORES = 8
P = 128
NE = EMB // P              # 8 contraction chunks
T = B * S                  # 4096 tokens
NT = T // P                # 32 token chunks
SB = S // P                # 16 key chunks per batch
QB = 512                   # query block (psum bank width)
NQ = S // QB               # 4 qblocks per batch
NBLK = T // QB             # 8 x-blocks
EXPF = mybir.ActivationFunctionType.Exp
SCALE = 1.0 / np.sqrt(HD)  # 0.125


def _build_nc(with_bv: bool, with_bo: bool):
    nc = bacc.Bacc(num_devices=N_CORES)
    dp = nc.declare_dram_parameter
    qT = dp("qT", [NBLK, P, NE, QB], BF, isOutput=False)
    kT = dp("kT", [NBLK, P, NE, QB], BF, isOutput=False)
    vT = dp("vT", [NBLK, P, NE, QB], BF, isOutput=False)
    WqT = dp("WqT", [P, NE, P], BF, isOutput=False)
    WkT = dp("WkT", [P, NE, P], BF, isOutput=False)
    WvT = dp("WvT", [P, NE, P], BF, isOutput=False)
    WoT = dp("WoT", [P, EMB], BF, isOutput=False)
    bqp = dp("bqp", [P, 1], F32, isOutput=False)
    bkp = dp("bkp", [P, 1], F32, isOutput=False)
    bvr = dp("bvr", [1, P], BF, isOutput=False)
    out = dp("out", [T, EMB], BF, isOutput=True)

    with tile.TileContext(nc) as tc, ExitStack() as ctx:
        wpool = ctx.enter_context(tc.tile_pool(name="wts", bufs=1))
        apool = ctx.enter_context(tc.tile_pool(name="acts", bufs=1))
        xpool = ctx.enter_context(tc.tile_pool(name="xin", bufs=6))
        ppool = ctx.enter_context(tc.tile_pool(name="probs", bufs=6))
        dpool = ctx.enter_context(tc.tile_pool(name="dn", bufs=4))
        opool = ctx.enter_context(tc.tile_pool(name="ob", bufs=3))
        genps = ctx.enter_context(tc.tile_pool(name="gen", bufs=2, space="PSUM"))
        scps = ctx.enter_context(tc.tile_pool(name="sc", bufs=2, space="PSUM"))
        attps = ctx.enter_context(tc.tile_pool(name="att", bufs=1, space="PSUM"))

        qhT = apool.tile([P, T], BF, tag="qhT")
        khT = apool.tile([P, T], BF, tag="khT")
        vhe = apool.tile([P, NT, 2, HD + 1], BF, tag="vhe")
        attT = apool.tile([P, T], BF, tag="attT")

        WqT_sb = wpool.tile([P, NE, P], BF, tag="WqT")
        WkT_sb = wpool.tile([P, NE, P], BF, tag="WkT")
        WvT_sb = wpool.tile([P, NE, P], BF, tag="WvT")
        WoT_sb = wpool.tile([P, EMB], BF, tag="WoT")
        bqp_sb = wpool.tile([P, 1], F32, tag="bqp")
        bkp_sb = wpool.tile([P, 1], F32, tag="bkp")
        warm = wpool.tile([P, QB], BF, tag="warm")
        dmy = wpool.tile([P, 8], BF, tag="dmy")
        nc.vector.memset(warm[:], 0.0)
        # dummy exp at t=0: the ~2.7us ACT table load runs under the DMA
        # staging shadow instead of stalling the first real exp
        nc.scalar.activation(dmy[:], warm[:, 0:8], EXPF, scale=SCALE)
        if with_bv:
            ones_b = wpool.tile([1, P], BF, tag="ones")
            nc.vector.memset(ones_b[:], 1.0)
            bvr_sb = wpool.tile([1, P], BF, tag="bvr")
            nc.sync.dma_start(bvr_sb[:], bvr[:])

        nc.vector.memset(vhe[:, :, :, HD], 1.0)

        # ---- DMA staging: one 8KiB-contiguous-per-partition DMA per block;
        # emitting all up front lets ring-slot WAR deps pace the prefetch ----
        nc.sync.dma_start(WqT_sb[:], WqT[:])
        nc.sync.dma_start(bqp_sb[:], bqp[:])
        nc.sync.dma_start(bkp_sb[:], bkp[:])
        nc.sync.dma_start(WkT_sb[:], WkT[:])
        nc.sync.dma_start(WvT_sb[:], WvT[:])
        xblocks = {}

        def stage_block(name, src, b, nb):
            xb = xpool.tile([P, NE, QB], BF, tag="xin", name=f"x_{name}_{b}_{nb}")
            nc.sync.dma_start(xb[:], src[b * NQ + nb, :, :, :])
            xblocks[(name, b, nb)] = xb

        # order: q-blk0 + all k of b0 (the minimum for the first score
        # quads), then v b0 (attention side), then q b0 rest, Wo, all of b1
        stage_block("q", qT, 0, 0)
        for nb in range(NQ):
            stage_block("k", kT, 0, nb)
        for nb in range(NQ):
            stage_block("v", vT, 0, nb)
        for nb in range(1, NQ):
            stage_block("q", qT, 0, nb)
        nc.sync.dma_start(WoT_sb[:], WoT[:])
        for name, src in (("q", qT), ("k", kT), ("v", vT)):
            for nb in range(NQ):
                stage_block(name, src, 1, nb)

        for _ in range(16):
            nc.tensor.matmul(
                genps.tile([P, QB], F32, tag="gen", name="warmps")[:],
                warm[:, 0:P], warm[:], start=True, stop=True,
            )

        # ---- emission helpers ----
        def qk_proj_half(dst, W_sb, xname, bias_sb, b, nb, half, ps_box):
            """Half of a 512-token projection block (4 of 8 k-chunks)."""
            t0 = b * S + nb * QB
            xb = xblocks[(xname, b, nb)]
            if half == 0:
                ps_box[0] = genps.tile(
                    [P, QB], F32, tag="gen", name=f"pj_{xname}_{b}_{nb}"
                )
            ps = ps_box[0]
            for kk in range(half * 4, half * 4 + 4):
                nc.tensor.matmul(
                    ps[:],
                    W_sb[:, kk, :],
                    xb[:, kk, :],
                    start=(kk == 0),
                    stop=(kk == NE - 1),
                )
            if half == 1:
                nc.vector.tensor_scalar_add(
                    dst[:, t0 : t0 + QB], ps[:], bias_sb[:, 0:1]
                )

        def qk_proj_unit(dst, W_sb, xname, bias_sb, b, nb):
            box = [None]
            qk_proj_half(dst, W_sb, xname, bias_sb, b, nb, 0, box)
            qk_proj_half(dst, W_sb, xname, bias_sb, b, nb, 1, box)

        def qk_halves(dst, W_sb, xname, bias_sb, b, nb):
            box = [None]
            return [
                (lambda h=h: qk_proj_half(dst, W_sb, xname, bias_sb, b, nb, h, box))
                for h in range(2)
            ]

        def vh_proj_unit(b, m):
            """Project one 128-token chunk of v into vhe (token-major)."""
            mm = b * SB + m
            xb = xblocks[("v", b, m // (QB // P))]
            mi = m % (QB // P)
            ps = genps.tile([P, QB], F32, tag="gen", name=f"pv_{mm}")
            for kk in range(NE):
                nc.tensor.matmul(
                    ps[:, 0:P],
                    xb[:, kk, mi * P : (mi + 1) * P],
                    WvT_sb[:, kk, :],
                    start=(kk == 0),
                    stop=(kk == NE - 1) and not with_bv,
                )
            if with_bv:
                nc.tensor.matmul(
                    ps[:, 0:P], ones_b[:], bvr_sb[:], start=False, stop=True
                )
            nc.vector.tensor_copy(
                vhe[:, mm, :, 0:HD], ps[:, 0:P].rearrange("p (h d) -> p h d", d=HD)
            )

        prio: deque = deque()     # normalize tails + out-proj: no DMA deps
        fillers: deque = deque()  # (min_step, fn): gated on x-block DMA arrival
        last_prio = [-10]

        def fill(step):
            # prio units carry DVE-heavy epilogue work; at most one per 2
            # steps so the DVE never backlogs the gen-psum ring (a backlog
            # stalls the in-order PE queue and gaps the exp pipeline)
            if prio and step - last_prio[0] >= 2:
                last_prio[0] = step
                prio.popleft()()
            elif fillers and fillers[0][0] <= step:
                fillers.popleft()[1]()

        def sc_chunk(b, qi, j):
            """Scores+exp for one key chunk: a 2x2 matmul quad, one ACT call."""
            q0 = b * S + qi * QB
            j0 = (b * SB + j) * P
            sc = scps.tile([P, 2 * QB], F32, tag="sc", name=f"sc_{b}_{qi}_{j}")
            nc.tensor.matmul(
                sc[0:HD, 0:QB],
                khT[0:HD, j0 : j0 + HD],
                qhT[0:HD, q0 : q0 + QB],
                start=True, stop=True,
            )
            nc.tensor.matmul(
                sc[HD:P, 0:QB],
                khT[0:HD, j0 + HD : j0 + P],
                qhT[0:HD, q0 : q0 + QB],
                start=True, stop=True,
            )
            nc.tensor.matmul(
                sc[0:HD, QB:],
                khT[HD:P, j0 : j0 + HD],
                qhT[HD:P, q0 : q0 + QB],
                start=True, stop=True,
            )
            nc.tensor.matmul(
                sc[HD:P, QB:],
                khT[HD:P, j0 + HD : j0 + P],
                qhT[HD:P, q0 : q0 + QB],
                start=True, stop=True,
            )
            pr = ppool.tile([P, 2 * QB], BF, tag="pr", name=f"pr_{b}_{qi}_{j}")
            nc.scalar.activation(pr[:], sc[:], EXPF, scale=SCALE)
            return pr

        def att_chunk(b, j, pr, attAB):
            jb = b * SB + j
            first = j == 0
            last = j == SB - 1
            nc.tensor.matmul(
                attAB[:, 0, :], vhe[:, jb, 0, :], pr[:, 0:QB], start=first, stop=last
            )
            nc.tensor.matmul(
                attAB[:, 1, :], vhe[:, jb, 1, :], pr[:, QB:], start=first, stop=last
            )

        def normalize_block(b, qi, attAB, last=False):
            # copy the accumulators out of PSUM in ONE op: the att psum slot
            # is WAR-waited by the NEXT qblock's first att matmul (in-order
            # PE queue!), so it must free fast, not after the full
            # broadcast/reciprocal/multiply chain
            q0 = b * S + qi * QB
            if last:
                # nothing reuses the att psum slot afterwards: skip the
                # staging copy and read PSUM directly (shorter drain chain)
                au2 = attAB
            else:
                au2 = dpool.tile(
                    [HD + 1, 2, QB], F32, tag="au", name=f"au_{b}_{qi}"
                )
                nc.vector.tensor_copy(au2[:], attAB[:])

            def tail(h):
                d0 = dpool.tile([1, QB], F32, tag=f"d0{h}", name=f"d0_{b}_{qi}_{h}")
                nc.vector.tensor_copy(d0[:], au2[HD : HD + 1, h, :])
                r0 = dpool.tile([1, QB], F32, tag=f"r0{h}", name=f"r0_{b}_{qi}_{h}")
                nc.vector.reciprocal_approx_fast(r0[:], d0[:])
                rb = dpool.tile([HD, QB], F32, tag=f"rb{h}", name=f"rb_{b}_{qi}_{h}")
                nc.gpsimd.partition_broadcast(rb[:], r0[:])
                nc.vector.tensor_mul(
                    attT[h * HD : (h + 1) * HD, q0 : q0 + QB],
                    au2[0:HD, h, :],
                    rb[:],
                )

            if last:
                tail(0)
                tail(1)
            else:
                prio.append(lambda: tail(0))
                prio.append(lambda: tail(1))

        tail_mode = [False]

        def outproj_unit(b, qi, mq):
            t0 = b * S + qi * QB + mq * P
            ob = opool.tile([P, EMB], BF, tag="ob", name=f"ob_{b}_{qi}_{mq}")
            for half in range(2):
                ps = genps.tile(
                    [P, QB], F32, tag="gen", name=f"o_{b}_{qi}_{mq}_{half}"
                )
                nc.tensor.matmul(
                    ps[:],
                    attT[:, t0 : t0 + P],
                    WoT_sb[:, half * QB : (half + 1) * QB],
                    start=True, stop=True,
                )
                if tail_mode[0]:
                    nc.scalar.copy(ob[:, half * QB : (half + 1) * QB], ps[:])
                else:
                    nc.vector.tensor_copy(
                        ob[:, half * QB : (half + 1) * QB], ps[:]
                    )
            nc.gpsimd.dma_start(out[t0 : t0 + P, :], ob[:])

        # ---- main schedule: minimal serial head, everything else fillers ----
        qk_proj_unit(qhT, WqT_sb, "q", bqp_sb, 0, 0)
        for nb in range(NQ):
            qk_proj_unit(khT, WkT_sb, "k", bkp_sb, 0, nb)

        # filler queue: (min_step, fn).  min_step approximates when the
        # unit's input DMA has landed (1 step ~ 1.1us of attention).
        for m in range(SB):
            fillers.append((3 * (m // 4), lambda m=m: vh_proj_unit(0, m)))
        for nb in range(1, NQ):
            fillers.append(
                (9 + 2 * nb,
                 lambda nb=nb: qk_proj_unit(qhT, WqT_sb, "q", bqp_sb, 0, nb))
            )
        for nb in range(NQ):
            for f in qk_halves(qhT, WqT_sb, "q", bqp_sb, 1, nb):
                fillers.append((26, f))
        for nb in range(NQ):
            for f in qk_halves(khT, WkT_sb, "k", bkp_sb, 1, nb):
                fillers.append((37, f))
        for m in range(SB):
            fillers.append((50 + 3 * (m // 4), lambda m=m: vh_proj_unit(1, m)))

        # flat attention stream: 128 chunks; the score quad of chunk i+1 is
        # emitted ahead of att(i-1) and fillers so exp(i+1) is ready the
        # moment exp(i) retires (the PE refills one 2-bank score buffer
        # while ACT drains the other)
        chunks = [
            (b, qi, j) for b in range(B) for qi in range(NQ) for j in range(SB)
        ]
        NCH = len(chunks)
        att_tiles = {}

        def get_att(b, qi):
            if (b, qi) not in att_tiles:
                att_tiles[(b, qi)] = attps.tile(
                    [HD + 1, 2, QB], F32, tag="att", name=f"att_{b}_{qi}"
                )
            return att_tiles[(b, qi)]

        sc_pr = {}

        def att_half(i, h):
            # one head's att matmul per step: halves the per-step PE load on
            # the critical chain so it fits the exp period even at cold clock
            b, qi, j = chunks[i]
            attAB = get_att(b, qi)
            pr = sc_pr[i]
            nc.tensor.matmul(
                attAB[:, h, :],
                vhe[:, b * SB + j, h, :],
                pr[:, h * QB : (h + 1) * QB],
                start=(j == 0),
                stop=(j == SB - 1),
            )
            if h == 1:
                sc_pr.pop(i)
                if j == SB - 1:
                    normalize_block(b, qi, attAB, last=(i == NCH - 1))
                    del att_tiles[(b, qi)]
                    for mq in range(QB // P):
                        prio.append(
                            lambda b=b, qi=qi, mq=mq: outproj_unit(b, qi, mq)
                        )

        sc_pr[0] = sc_chunk(*chunks[0])
        sc_pr[1] = sc_chunk(*chunks[1])
        fill(0)  # vhe chunk 0 before att(0)
        for i in range(1, NCH):
            if i + 1 < NCH:
                sc_pr[i + 1] = sc_chunk(*chunks[i + 1])
            if i >= 2:
                att_half(i - 2, 1)   # older chunk's head B first (ordering!)
            att_half(i - 1, 0)
            fill(i)
            if i <= 16:
                fill(i)
        tail_mode[0] = True
        att_half(NCH - 2, 1)
        att_half(NCH - 1, 0)
        att_half(NCH - 1, 1)
        for _ in range(4):
            nc.tensor.matmul(
                genps.tile([P, QB], F32, tag="gen", name="warmtail")[:],
                warm[:, 0:P], warm[:], start=True, stop=True,
            )
        while prio:
            prio.popleft()()
        while fillers:
            fillers.popleft()[1]()

    nc.finalize()
    return nc


_NC_CACHE: dict = {}
_BO_CACHE: list = [None]


def _get_nc(with_bv: bool, with_bo: bool):
    key = (with_bv, with_bo)
    if key not in _NC_CACHE:
        _NC_CACHE[key] = _build_nc(*key)
    return _NC_CACHE[key]


def _feat_tiled(xT):
    """[EMB, n] -> [128, NE, n] contiguous (feature chunks on partitions)."""
    n = xT.shape[1]
    return np.ascontiguousarray(xT.reshape(NE, P, n).transpose(1, 0, 2))


def _stage(inputs):
    bf = ml_dtypes.bfloat16
    f32 = np.float32

    def arr(name):
        return np.asarray(inputs[name], f32)

    q, k, v = arr("q"), arr("k"), arr("v")
    Wq, Wk, Wv, Wo = arr("Wq"), arr("Wk"), arr("Wv"), arr("Wo")
    bq, bk, bv, bo = arr("bq"), arr("bk"), arr("bv"), arr("bo")

    with_bv = bool(np.any(bv))
    with_bo = bool(np.any(bo))
    _BO_CACHE[0] = bo if with_bo else None

    def xt(x3d):  # [B,S,EMB] -> [NBLK, 128, NE, QB] bf16, blocked contiguous
        xT = np.ascontiguousarray(x3d.reshape(T, EMB).T)  # [EMB, T]
        blocks = [
            _feat_tiled(xT[:, i * QB : (i + 1) * QB]) for i in range(NBLK)
        ]
        return np.ascontiguousarray(np.stack(blocks)).astype(bf)

    qTt, kTt, vTt = xt(q), xt(k), xt(v)

    in_maps = []
    for c in range(N_CORES):
        F = slice(c * P, (c + 1) * P)
        m = {
            "qT": qTt,
            "kT": kTt,
            "vT": vTt,
            "WqT": _feat_tiled(np.ascontiguousarray(Wq.T[:, F])).astype(bf),
            "WkT": _feat_tiled(np.ascontiguousarray(Wk.T[:, F])).astype(bf),
            "WvT": _feat_tiled(np.ascontiguousarray(Wv.T[:, F])).astype(bf),
            "WoT": np.ascontiguousarray(Wo.T[F, :]).astype(bf),
            "bqp": np.ascontiguousarray(bq[F][:, None]),
            "bkp": np.ascontiguousarray(bk[F][:, None]),
            "bvr": np.ascontiguousarray(bv[F][None, :]).astype(bf),
        }
        in_maps.append(m)
    return in_maps, with_bv, with_bo


def _assemble(results):
    acc = results[0]["out"].astype(np.float32)
    for c in range(1, N_CORES):
        acc += results[c]["out"].astype(np.float32)
    if _BO_CACHE[0] is not None:
        acc += _BO_CACHE[0]
    return acc.reshape(B, S, EMB)


def kernel(**inputs) -> np.ndarray:
    in_maps, with_bv, with_bo = _stage(inputs)
    nc = _get_nc(with_bv, with_bo)
    res = run_bass_kernel_spmd(nc, in_maps, list(range(N_CORES)))
    return _assemble(res.results)



# revision 12
# speedup vs baseline: 1.0353x; 1.0078x over previous
"""Multi-head attention (B=2, S=2048, EMB=1024, H=16) on 8 Trainium2 cores.

v3 — tensor-parallel over heads: core c owns heads {2c, 2c+1} (a 128-wide
feature slice F_c of the QKV projections and of Wo's rows).  Each core:
  - projects q/k/v for ALL 4096 tokens through its 1024x128 weight slices
  - runs attention for its 2 heads over both batches
  - computes the PARTIAL output projection out_c = att_c @ Wo[:, F_c].T
The 8 partial outputs (bf16) are summed on the host (linear in Wo rows), so
no device collectives are needed and no compute is redundant: per-core FLOPs
drop from ~15 G (query-sharded baseline with 4x-redundant K/V projections)
to ~8.6 G (perfect 1/8 split of total work).

Layouts (all device matmuls contract over the partition dim):
  - inputs staged HOST-side as [8 blocks, 128, 8, 512] so every x-block DMA
    is one 8 KiB-contiguous run per partition (descriptor count, not
    bandwidth, limited an earlier cut)
  - qhT/khT [128 = (headA 64 | headB 64), 4096 tok] bf16, feature-major
  - vhe [128 tok, chunk, head, 65] bf16 token-major with an all-ones 65th
    column so the att matmul accumulates softmax denominators for free
  - scores_T [k, q] per key-chunk via a 2x2 tile_position quad: 4 matmuls
    (2 heads x 2 key-halves, K=64 M=64 N=512) occupy disjoint PE-array
    quadrants and run concurrently (measured 54 ns/MM vs 217 serial)
  - exp: one ACT call per chunk ([128, 1024] over a 2-bank PSUM tile,
    scale=1/8, no max subtraction; |scores|/8 < ~7 so exp stays in range)
  - normalize: copy denom row, gpsimd partition_broadcast, DVE
    reciprocal_approx_fast on [64, 512] (full-width lanes), multiply
  - out partial [tok, 1024] bf16: one DMA per 128-token chunk, issued from
    the gpsimd queue to keep descriptor generation off the sync engine

Pipelining: the score PSUM is double-buffered (scps bufs=2), so the quad of
chunk j+1 fills one 2-bank buffer while ACT exp-drains the other — ACT runs
gapless and the PE never idles long enough for the HAM clock gate to drop
the array to 1.2 GHz (the dominant loss in earlier cuts).  All other PE work
(v-projection of batch 0, every projection of batch 1, the out-projection of
the previous qblock) is queued as <=1.2us filler units, popped one per chunk
into the PE's ~0.5us-per-chunk slack.
"""

from collections import deque
from contextlib import ExitStack

import numpy as np
import ml_dtypes

import concourse.bass as bass  # noqa: F401
import concourse.mybir as mybir
import concourse.tile as tile
from concourse import bacc
from concourse.bass_utils import run_bass_kernel_spmd

BF = mybir.dt.bfloat16
F32 = mybir.dt.float32

EMB = 1024
HEADS = 16
HD = EMB // HEADS          # 64
B, S = 2, 2048
N_CORES = 8
P = 128
NE = EMB // P              # 8 contraction chunks
T = B * S                  # 4096 tokens
NT = T // P                # 32 token chunks
SB = S // P                # 16 key chunks per batch
QB = 512                   # query block (psum bank width)
NQ = S // QB               # 4 qblocks per batch
NBLK = T // QB             # 8 x-blocks
EXPF = mybir.ActivationFunctionType.Exp
SCALE = 1.0 / np.sqrt(HD)  # 0.125


def _build_nc(with_bv: bool, with_bo: bool):
    nc = bacc.Bacc(num_devices=N_CORES)
    dp = nc.declare_dram_parameter
    qT = dp("qT", [NBLK, P, NE, QB], BF, isOutput=False)
    kT = dp("kT", [NBLK, P, NE, QB], BF, isOutput=False)
    vT = dp("vT", [NBLK, P, NE, QB], BF, isOutput=False)
    WqT = dp("WqT", [P, NE, P], BF, isOutput=False)
    WkT = dp("WkT", [P, NE, P], BF, isOutput=False)
    WvT = dp("WvT", [P, NE, P], BF, isOutput=False)
    WoT = dp("WoT", [P, EMB], BF, isOutput=False)
    bqp = dp("bqp", [P, 1], F32, isOutput=False)
    bkp = dp("bkp", [P, 1], F32, isOutput=False)
    bvr = dp("bvr", [1, P], BF, isOutput=False)
    out = dp("out", [T, EMB], BF, isOutput=True)

    with tile.TileContext(nc) as tc, ExitStack() as ctx:
        wpool = ctx.enter_context(tc.tile_pool(name="wts", bufs=1))
        apool = ctx.enter_context(tc.tile_pool(name="acts", bufs=1))
        xpool = ctx.enter_context(tc.tile_pool(name="xin", bufs=6))
        ppool = ctx.enter_context(tc.tile_pool(name="probs", bufs=6))
        dpool = ctx.enter_context(tc.tile_pool(name="dn", bufs=4))
        opool = ctx.enter_context(tc.tile_pool(name="ob", bufs=3))
        genps = ctx.enter_context(tc.tile_pool(name="gen", bufs=2, space="PSUM"))
        scps = ctx.enter_context(tc.tile_pool(name="sc", bufs=2, space="PSUM"))
        attps = ctx.enter_context(tc.tile_pool(name="att", bufs=1, space="PSUM"))

        qhT = apool.tile([P, T], BF, tag="qhT")
        khT = apool.tile([P, T], BF, tag="khT")
        vhe = apool.tile([P, NT, 2, HD + 1], BF, tag="vhe")
        attT = apool.tile([P, T], BF, tag="attT")

        WqT_sb = wpool.tile([P, NE, P], BF, tag="WqT")
        WkT_sb = wpool.tile([P, NE, P], BF, tag="WkT")
        WvT_sb = wpool.tile([P, NE, P], BF, tag="WvT")
        WoT_sb = wpool.tile([P, EMB], BF, tag="WoT")
        bqp_sb = wpool.tile([P, 1], F32, tag="bqp")
        bkp_sb = wpool.tile([P, 1], F32, tag="bkp")
        warm = wpool.tile([P, QB], BF, tag="warm")
        dmy = wpool.tile([P, 8], BF, tag="dmy")
        nc.vector.memset(warm[:], 0.0)
        # dummy exp at t=0: the ~2.7us ACT table load runs under the DMA
        # staging shadow instead of stalling the first real exp
        nc.scalar.activation(dmy[:], warm[:, 0:8], EXPF, scale=SCALE)
        if with_bv:
            ones_b = wpool.tile([1, P], BF, tag="ones")
            nc.vector.memset(ones_b[:], 1.0)
            bvr_sb = wpool.tile([1, P], BF, tag="bvr")
            nc.sync.dma_start(bvr_sb[:], bvr[:])

        nc.vector.memset(vhe[:, :, :, HD], 1.0)

        # ---- DMA staging: one 8KiB-contiguous-per-partition DMA per block;
        # emitting all up front lets ring-slot WAR deps pace the prefetch ----
        nc.sync.dma_start(WqT_sb[:], WqT[:])
        nc.sync.dma_start(bqp_sb[:], bqp[:])
        nc.sync.dma_start(bkp_sb[:], bkp[:])
        nc.sync.dma_start(WkT_sb[:], WkT[:])
        nc.sync.dma_start(WvT_sb[:], WvT[:])
        xblocks = {}

        def stage_block(name, src, b, nb, split=False):
            xb = xpool.tile([P, NE, QB], BF, tag="xin", name=f"x_{name}_{b}_{nb}")
            if split:
                # kk-half granularity: the first projections pipeline
                # behind the DMA via Tile's subtile dependency tracking
                nc.sync.dma_start(xb[:, 0:4, :], src[b * NQ + nb, :, 0:4, :])
                nc.sync.dma_start(xb[:, 4:8, :], src[b * NQ + nb, :, 4:8, :])
            else:
                nc.sync.dma_start(xb[:], src[b * NQ + nb, :, :, :])
            xblocks[(name, b, nb)] = xb

        # order: q-blk0 + all k of b0 (the minimum for the first score
        # quads), then v b0 (attention side), then q b0 rest, Wo, all of b1
        stage_block("q", qT, 0, 0, split=True)
        stage_block("k", kT, 0, 0, split=True)
        for nb in range(1, NQ):
            stage_block("k", kT, 0, nb)
        for nb in range(NQ):
            stage_block("v", vT, 0, nb)
        for nb in range(1, NQ):
            stage_block("q", qT, 0, nb)
        nc.sync.dma_start(WoT_sb[:], WoT[:])
        for name, src in (("q", qT), ("k", kT), ("v", vT)):
            for nb in range(NQ):
                stage_block(name, src, 1, nb)

        for _ in range(6):
            nc.tensor.matmul(
                genps.tile([P, QB], F32, tag="gen", name="warmps")[:],
                warm[:, 0:P], warm[:], start=True, stop=True,
            )

        # ---- emission helpers ----
        def qk_proj_half(dst, W_sb, xname, bias_sb, b, nb, half, ps_box):
            """Half of a 512-token projection block (4 of 8 k-chunks)."""
            t0 = b * S + nb * QB
            xb = xblocks[(xname, b, nb)]
            if half == 0:
                ps_box[0] = genps.tile(
                    [P, QB], F32, tag="gen", name=f"pj_{xname}_{b}_{nb}"
                )
            ps = ps_box[0]
            for kk in range(half * 4, half * 4 + 4):
                nc.tensor.matmul(
                    ps[:],
                    W_sb[:, kk, :],
                    xb[:, kk, :],
                    start=(kk == 0),
                    stop=(kk == NE - 1),
                )
            if half == 1:
                nc.vector.tensor_scalar_add(
                    dst[:, t0 : t0 + QB], ps[:], bias_sb[:, 0:1]
                )

        def qk_proj_unit(dst, W_sb, xname, bias_sb, b, nb):
            box = [None]
            qk_proj_half(dst, W_sb, xname, bias_sb, b, nb, 0, box)
            qk_proj_half(dst, W_sb, xname, bias_sb, b, nb, 1, box)

        def qk_halves(dst, W_sb, xname, bias_sb, b, nb):
            box = [None]
            return [
                (lambda h=h: qk_proj_half(dst, W_sb, xname, bias_sb, b, nb, h, box))
                for h in range(2)
            ]

        def vh_proj_unit(b, m):
            """Project one 128-token chunk of v into vhe (token-major)."""
            mm = b * SB + m
            xb = xblocks[("v", b, m // (QB // P))]
            mi = m % (QB // P)
            ps = genps.tile([P, QB], F32, tag="gen", name=f"pv_{mm}")
            for kk in range(NE):
                nc.tensor.matmul(
                    ps[:, 0:P],
                    xb[:, kk, mi * P : (mi + 1) * P],
                    WvT_sb[:, kk, :],
                    start=(kk == 0),
                    stop=(kk == NE - 1) and not with_bv,
                )
            if with_bv:
                nc.tensor.matmul(
                    ps[:, 0:P], ones_b[:], bvr_sb[:], start=False, stop=True
                )
            nc.vector.tensor_copy(
                vhe[:, mm, :, 0:HD], ps[:, 0:P].rearrange("p (h d) -> p h d", d=HD)
            )

        prio: deque = deque()     # normalize tails + out-proj: no DMA deps
        fillers: deque = deque()  # (min_step, fn): gated on x-block DMA arrival
        last_prio = [-10]

        def fill(step):
            # prio units carry DVE-heavy epilogue work; at most one per 2
            # steps so the DVE never backlogs the gen-psum ring (a backlog
            # stalls the in-order PE queue and gaps the exp pipeline)
            if prio and step - last_prio[0] >= 2:
                last_prio[0] = step
                prio.popleft()()
            elif fillers and fillers[0][0] <= step:
                fillers.popleft()[1]()

        def sc_chunk(b, qi, j):
            """Scores+exp for one key chunk: a 2x2 matmul quad, one ACT call."""
            q0 = b * S + qi * QB
            j0 = (b * SB + j) * P
            sc = scps.tile([P, 2 * QB], F32, tag="sc", name=f"sc_{b}_{qi}_{j}")
            nc.tensor.matmul(
                sc[0:HD, 0:QB],
                khT[0:HD, j0 : j0 + HD],
                qhT[0:HD, q0 : q0 + QB],
                start=True, stop=True,
            )
            nc.tensor.matmul(
                sc[HD:P, 0:QB],
                khT[0:HD, j0 + HD : j0 + P],
                qhT[0:HD, q0 : q0 + QB],
                start=True, stop=True,
            )
            nc.tensor.matmul(
                sc[0:HD, QB:],
                khT[HD:P, j0 : j0 + HD],
                qhT[HD:P, q0 : q0 + QB],
                start=True, stop=True,
            )
            nc.tensor.matmul(
                sc[HD:P, QB:],
                khT[HD:P, j0 + HD : j0 + P],
                qhT[HD:P, q0 : q0 + QB],
                start=True, stop=True,
            )
            pr = ppool.tile([P, 2 * QB], BF, tag="pr", name=f"pr_{b}_{qi}_{j}")
            nc.scalar.activation(pr[:], sc[:], EXPF, scale=SCALE)
            return pr

        def att_chunk(b, j, pr, attAB):
            jb = b * SB + j
            first = j == 0
            last = j == SB - 1
            nc.tensor.matmul(
                attAB[:, 0, :], vhe[:, jb, 0, :], pr[:, 0:QB], start=first, stop=last
            )
            nc.tensor.matmul(
                attAB[:, 1, :], vhe[:, jb, 1, :], pr[:, QB:], start=first, stop=last
            )

        def normalize_block(b, qi, attAB, last=False):
            # copy the accumulators out of PSUM in ONE op: the att psum slot
            # is WAR-waited by the NEXT qblock's first att matmul (in-order
            # PE queue!), so it must free fast, not after the full
            # broadcast/reciprocal/multiply chain
            q0 = b * S + qi * QB
            if last:
                # nothing reuses the att psum slot afterwards: skip the
                # staging copy and read PSUM directly (shorter drain chain)
                au2 = attAB
            else:
                au2 = dpool.tile(
                    [HD + 1, 2, QB], F32, tag="au", name=f"au_{b}_{qi}"
                )
                nc.vector.tensor_copy(au2[:], attAB[:])

            def tail(h):
                d0 = dpool.tile([1, QB], F32, tag=f"d0{h}", name=f"d0_{b}_{qi}_{h}")
                nc.vector.tensor_copy(d0[:], au2[HD : HD + 1, h, :])
                r0 = dpool.tile([1, QB], F32, tag=f"r0{h}", name=f"r0_{b}_{qi}_{h}")
                nc.vector.reciprocal_approx_fast(r0[:], d0[:])
                rb = dpool.tile([HD, QB], F32, tag=f"rb{h}", name=f"rb_{b}_{qi}_{h}")
                nc.gpsimd.partition_broadcast(rb[:], r0[:])
                nc.vector.tensor_mul(
                    attT[h * HD : (h + 1) * HD, q0 : q0 + QB],
                    au2[0:HD, h, :],
                    rb[:],
                )

            if last:
                tail(0)
                tail(1)
            else:
                prio.append(lambda: tail(0))
                prio.append(lambda: tail(1))

        tail_mode = [False]

        def outproj_unit(b, qi, mq):
            t0 = b * S + qi * QB + mq * P
            ob = opool.tile([P, EMB], BF, tag="ob", name=f"ob_{b}_{qi}_{mq}")
            for half in range(2):
                ps = genps.tile(
                    [P, QB], F32, tag="gen", name=f"o_{b}_{qi}_{mq}_{half}"
                )
                nc.tensor.matmul(
                    ps[:],
                    attT[:, t0 : t0 + P],
                    WoT_sb[:, half * QB : (half + 1) * QB],
                    start=True, stop=True,
                )
                if tail_mode[0]:
                    nc.scalar.copy(ob[:, half * QB : (half + 1) * QB], ps[:])
                else:
                    nc.vector.tensor_copy(
                        ob[:, half * QB : (half + 1) * QB], ps[:]
                    )
            nc.gpsimd.dma_start(out[t0 : t0 + P, :], ob[:])

        # ---- main schedule: minimal serial head, everything else fillers ----
        qk_proj_unit(qhT, WqT_sb, "q", bqp_sb, 0, 0)
        for nb in range(NQ):
            qk_proj_unit(khT, WkT_sb, "k", bkp_sb, 0, nb)

        # filler queue: (min_step, fn).  min_step approximates when the
        # unit's input DMA has landed (1 step ~ 1.1us of attention).
        for m in range(SB):
            fillers.append((3 * (m // 4), lambda m=m: vh_proj_unit(0, m)))
        for nb in range(1, NQ):
            fillers.append(
                (9 + 2 * nb,
                 lambda nb=nb: qk_proj_unit(qhT, WqT_sb, "q", bqp_sb, 0, nb))
            )
        for nb in range(NQ):
            for f in qk_halves(qhT, WqT_sb, "q", bqp_sb, 1, nb):
                fillers.append((26, f))
        for nb in range(NQ):
            for f in qk_halves(khT, WkT_sb, "k", bkp_sb, 1, nb):
                fillers.append((37, f))
        for m in range(SB):
            fillers.append((50 + 3 * (m // 4), lambda m=m: vh_proj_unit(1, m)))

        # flat attention stream: 128 chunks; the score quad of chunk i+1 is
        # emitted ahead of att(i-1) and fillers so exp(i+1) is ready the
        # moment exp(i) retires (the PE refills one 2-bank score buffer
        # while ACT drains the other)
        chunks = [
            (b, qi, j) for b in range(B) for qi in range(NQ) for j in range(SB)
        ]
        NCH = len(chunks)
        att_tiles = {}

        def get_att(b, qi):
            if (b, qi) not in att_tiles:
                att_tiles[(b, qi)] = attps.tile(
                    [HD + 1, 2, QB], F32, tag="att", name=f"att_{b}_{qi}"
                )
            return att_tiles[(b, qi)]

        sc_pr = {}

        def att_half(i, h):
            # one head's att matmul per step: halves the per-step PE load on
            # the critical chain so it fits the exp period even at cold clock
            b, qi, j = chunks[i]
            attAB = get_att(b, qi)
            pr = sc_pr[i]
            nc.tensor.matmul(
                attAB[:, h, :],
                vhe[:, b * SB + j, h, :],
                pr[:, h * QB : (h + 1) * QB],
                start=(j == 0),
                stop=(j == SB - 1),
            )
            if h == 1:
                sc_pr.pop(i)
                if j == SB - 1:
                    normalize_block(b, qi, attAB, last=(i == NCH - 1))
                    del att_tiles[(b, qi)]
                    for mq in range(QB // P):
                        prio.append(
                            lambda b=b, qi=qi, mq=mq: outproj_unit(b, qi, mq)
                        )

        sc_pr[0] = sc_chunk(*chunks[0])
        sc_pr[1] = sc_chunk(*chunks[1])
        fill(0)  # vhe chunk 0 before att(0)
        for i in range(1, NCH):
            if i + 1 < NCH:
                sc_pr[i + 1] = sc_chunk(*chunks[i + 1])
            if i >= 2:
                att_half(i - 2, 1)   # older chunk's head B first (ordering!)
            att_half(i - 1, 0)
            fill(i)
            if i <= 16:
                fill(i)
        tail_mode[0] = True
        att_half(NCH - 2, 1)
        att_half(NCH - 1, 0)
        att_half(NCH - 1, 1)
        for _ in range(4):
            nc.tensor.matmul(
                genps.tile([P, QB], F32, tag="gen", name="warmtail")[:],
                warm[:, 0:P], warm[:], start=True, stop=True,
            )
        while prio:
            prio.popleft()()
        while fillers:
            fillers.popleft()[1]()

    nc.finalize()
    return nc


_NC_CACHE: dict = {}
_BO_CACHE: list = [None]


def _get_nc(with_bv: bool, with_bo: bool):
    key = (with_bv, with_bo)
    if key not in _NC_CACHE:
        _NC_CACHE[key] = _build_nc(*key)
    return _NC_CACHE[key]


def _feat_tiled(xT):
    """[EMB, n] -> [128, NE, n] contiguous (feature chunks on partitions)."""
    n = xT.shape[1]
    return np.ascontiguousarray(xT.reshape(NE, P, n).transpose(1, 0, 2))


def _stage(inputs):
    bf = ml_dtypes.bfloat16
    f32 = np.float32

    def arr(name):
        return np.asarray(inputs[name], f32)

    q, k, v = arr("q"), arr("k"), arr("v")
    Wq, Wk, Wv, Wo = arr("Wq"), arr("Wk"), arr("Wv"), arr("Wo")
    bq, bk, bv, bo = arr("bq"), arr("bk"), arr("bv"), arr("bo")

    with_bv = bool(np.any(bv))
    with_bo = bool(np.any(bo))
    _BO_CACHE[0] = bo if with_bo else None

    def xt(x3d):  # [B,S,EMB] -> [NBLK, 128, NE, QB] bf16, blocked contiguous
        xT = np.ascontiguousarray(x3d.reshape(T, EMB).T)  # [EMB, T]
        blocks = [
            _feat_tiled(xT[:, i * QB : (i + 1) * QB]) for i in range(NBLK)
        ]
        return np.ascontiguousarray(np.stack(blocks)).astype(bf)

    qTt, kTt, vTt = xt(q), xt(k), xt(v)

    in_maps = []
    for c in range(N_CORES):
        F = slice(c * P, (c + 1) * P)
        m = {
            "qT": qTt,
            "kT": kTt,
            "vT": vTt,
            "WqT": _feat_tiled(np.ascontiguousarray(Wq.T[:, F])).astype(bf),
            "WkT": _feat_tiled(np.ascontiguousarray(Wk.T[:, F])).astype(bf),
            "WvT": _feat_tiled(np.ascontiguousarray(Wv.T[:, F])).astype(bf),
            "WoT": np.ascontiguousarray(Wo.T[F, :]).astype(bf),
            "bqp": np.ascontiguousarray(bq[F][:, None]),
            "bkp": np.ascontiguousarray(bk[F][:, None]),
            "bvr": np.ascontiguousarray(bv[F][None, :]).astype(bf),
        }
        in_maps.append(m)
    return in_maps, with_bv, with_bo


def _assemble(results):
    acc = results[0]["out"].astype(np.float32)
    for c in range(1, N_CORES):
        acc += results[c]["out"].astype(np.float32)
    if _BO_CACHE[0] is not None:
        acc += _BO_CACHE[0]
    return acc.reshape(B, S, EMB)


def kernel(**inputs) -> np.ndarray:
    in_maps, with_bv, with_bo = _stage(inputs)
    nc = _get_nc(with_bv, with_bo)
    res = run_bass_kernel_spmd(nc, in_maps, list(range(N_CORES)))
    return _assemble(res.results)

